# revision 10
# baseline (speedup 1.0000x reference)
"""Trainium2 Bass kernel for the CANN recurrent-net problem.

Computation (reference): 14 sequential steps of
    temp   = J @ r                      (J: 1680x1680 fp32)
    U      = temp + Iext
    sq     = (0.2*U)^2 = 0.04*U^2
    recSum = sum(0.005*sq)
    r_new  = sq / recSum
returning (U_last, recSum_last, r_last).

Kernel strategy (single-core latency problem; replicated across the 8 cores
per the sharding hint -- a per-step 8-core allgather floor of ~4.6us x 14
steps would cost more than the whole replicated compute):
  * Keep J'^T = (200*J).T resident in SBUF (zero-padded to [1792, 2048]).
  * Deferred normalization: propagate w = U^2 (unnormalized) as the state.
    J @ r_new = (J' @ w) * (1/sum(w)) by linearity, applied as the ACT-copy
    scale (a [1,1] per-partition scalar at partition 0). Initial state
    w0 = r0 with scale 0.005 (cancels the 200 baked into J').
  * Matvec per step on the PE: w-column tiles [128,1] stationary,
    J'^T tiles [128, 512] moving -> psum [1, 2048] accumulated over 14
    K-tiles (stream-bound, ~1 col/cycle).
  * psum [1,2048] -> ACT copy (applies 1/sum(w)) -> sbuf temp
    -> 16 PE transpose-mode ops [1,128]->[128,1] -> [128,16] layout.
  * Elementwise in [128,14] layout (cheap): U_T = psum_T + Iext_T,
    w = U_T^2 with fused free-dim reduce (tensor_tensor_reduce),
    128->1 partition reduce via ones-matmul, reciprocal on DVE.

Device outputs: U_T [128,14] and recSum [1,1]; the host derives
r_last = 0.04*U^2/recSum (1680 elements, negligible) and reshapes.
Padding: rows/cols >= 1680 are zero in J', Iext and w0, so padded lanes
stay exactly zero through every step and never affect sums.
"""

import sys

if "/opt/trn_rl_repo" not in sys.path:
    sys.path.insert(0, "/opt/trn_rl_repo")

import numpy as np

import concourse.bacc as bacc
import concourse.mybir as mybir
import concourse.tile as tile
from concourse.bass_utils import run_bass_kernel_spmd

R, C = 30, 56
N = R * C  # 1680
KT = 14  # K tiles of 128 (1792 padded)
NT = 16  # N transpose columns of 128 (2048 padded)
KPAD = KT * 128  # 1792
NPAD = NT * 128  # 2048
STEPS = 14  # PRCN - 1
NCHUNK = 512  # psum bank = 512 fp32
F32 = mybir.dt.float32

_CACHE = {}


def _build_bass():
    nc = bacc.Bacc(
        "TRN2", target_bir_lowering=False, debug=False, num_devices=8
    )

    jt = nc.dram_tensor("jt", [KT, 128, NPAD], F32, kind="ExternalInput")
    w0 = nc.dram_tensor("w0", [128, KT], F32, kind="ExternalInput")
    iext = nc.dram_tensor("iext", [128, KT], F32, kind="ExternalInput")
    s0 = nc.dram_tensor("s0", [1, 1], F32, kind="ExternalInput")
    ones_col = nc.dram_tensor("ones_col", [128, 1], F32, kind="ExternalInput")
    one11 = nc.dram_tensor("one11", [1, 1], F32, kind="ExternalInput")

    u_out = nc.dram_tensor("u_out", [128, KT], F32, kind="ExternalOutput")
    sum_out = nc.dram_tensor("sum_out", [1, 1], F32, kind="ExternalOutput")

    with tile.TileContext(nc) as tc:
        with (
            tc.tile_pool(name="jpool", bufs=1) as jpool,
            tc.tile_pool(name="sm", bufs=1) as sm,
            tc.tile_pool(name="ps", bufs=1, space="PSUM") as ps,
        ):
            jt_sb = jpool.tile([128, KT, NPAD], F32)
            temp = sm.tile([1, NPAD], F32)
            w = sm.tile([128, KT], F32)
            u_sb = sm.tile([128, KT], F32)
            iext_sb = sm.tile([128, KT], F32)
            rinv = sm.tile([1, 1], F32)
            partial = sm.tile([128, 1], F32)
            ones_col_sb = sm.tile([128, 1], F32)
            one11_sb = sm.tile([1, 1], F32)
            sums_sb = sm.tile([1, 1], F32)

            psum_mv = ps.tile([128, NPAD], F32)  # banks 0-3 (chunks on part 0)
            psum_t = ps.tile([128, NT], F32)  # transposed result
            psum_s = ps.tile([1, 1], F32)  # scalar sum
            psum_x = ps.tile([1, 1], F32)  # scratch for PE priming

            # Input loads (per-K-tile so step-1 matmuls can start early).
            for k in range(KT):
                nc.sync.dma_start(jt_sb[:, k, :], jt[k])
            nc.sync.dma_start(w[:], w0[:])
            nc.sync.dma_start(iext_sb[:], iext[:])
            nc.sync.dma_start(rinv[:], s0[:])
            nc.sync.dma_start(ones_col_sb[:], ones_col[:])
            nc.sync.dma_start(one11_sb[:], one11[:])

            # PE "priming" matmuls: the HW LDWEIGHTS slot fits only ONE sync
            # wait; touch each DMA-loaded constant once with a single-dep
            # dummy matmul so PE's observed semaphore ticks already cover
            # the DMA before the real (compute-dependent) uses.
            for lhsT, rhs in (
                (w[:, 0:1], w[:, 0:1]),
                (ones_col_sb[:, 0:1], ones_col_sb[:, 0:1]),
                (one11_sb[0:1, 0:1], one11_sb[0:1, 0:1]),
            ):
                nc.tensor.matmul(
                    psum_x[0:1, 0:1], lhsT, rhs, start=True, stop=True
                )

            for step in range(STEPS):
                # ---- matvec: psum_mv[0, :] = J'^T.T @ w  (accum over K) ----
                for c in range(NPAD // NCHUNK):
                    cs = c * NCHUNK
                    for k in range(KT):
                        nc.tensor.matmul(
                            psum_mv[0:1, cs : cs + NCHUNK],
                            w[:, k : k + 1],
                            jt_sb[:, k, cs : cs + NCHUNK],
                            start=(k == 0),
                            stop=(k == KT - 1),
                        )

                # ---- scaled copy psum -> sbuf (applies 1/sum(w)) ----
                # One ACT per PSUM bank: a single AP must not span banks.
                for c in range(NPAD // NCHUNK):
                    cs = c * NCHUNK
                    nc.scalar.activation(
                        temp[0:1, cs : cs + NCHUNK],
                        psum_mv[0:1, cs : cs + NCHUNK],
                        mybir.ActivationFunctionType.Copy,
                        scale=rinv[0:1, :],
                    )

                # ---- transpose [1,2048] -> [128,16] ----
                # Outer-product matmuls (lhsT=[1,128], rhs=[[1.0]]), NOT
                # transpose-mode: is_transpose after fp32 matmuls hangs the
                # HW (NRT_EXEC_UNIT_UNRECOVERABLE, bisected on silicon).
                for i in range(NT):
                    nc.tensor.matmul(
                        psum_t[:, i : i + 1],
                        temp[0:1, i * 128 : (i + 1) * 128],
                        one11_sb[0:1, :],
                        start=True,
                        stop=True,
                    )

                # ---- U = temp_T + Iext ; w = U^2 ; partial = sum_free(w) ----
                nc.vector.tensor_add(u_sb[:, :], psum_t[:, :KT], iext_sb[:, :])
                # ACT Square with fused per-partition accumulate. (The DVE
                # tensor_tensor_reduce equivalent crashes the device --
                # bisected on silicon.)
                nc.scalar.activation(
                    w[:, :],
                    u_sb[:, :],
                    mybir.ActivationFunctionType.Square,
                    accum_out=partial[:, :],
                )

                # ---- sum over partitions -> 1/x ----
                nc.tensor.matmul(
                    psum_s[0:1, 0:1], ones_col_sb[:, :], partial[:, :],
                    start=True, stop=True,
                )
                nc.vector.reciprocal(rinv[0:1, :], psum_s[0:1, :])

            # ---- epilogue: recSum = 2e-4 * sum(w_last) ----
            nc.scalar.activation(
                sums_sb[0:1, :], psum_s[0:1, :],
                mybir.ActivationFunctionType.Copy, scale=2.0e-4,
            )

            nc.sync.dma_start(u_out[:], u_sb[:, :])
            nc.sync.dma_start(sum_out[:], sums_sb[:, :])

    if not nc.is_finalized():
        nc.finalize()
    return nc


def _prep_inputs(net_in: np.ndarray, J: np.ndarray):
    net_in = np.asarray(net_in, dtype=np.float32)
    J = np.asarray(J, dtype=np.float32)

    jt_pad = np.zeros((KPAD, NPAD), dtype=np.float32)
    jt_pad[:N, :N] = (np.float32(200.0) * J).T
    jt_pad = np.ascontiguousarray(jt_pad.reshape(KT, 128, NPAD))

    def to_T(v):
        p = np.zeros(KPAD, dtype=np.float32)
        p[:N] = v
        return np.ascontiguousarray(p.reshape(KT, 128).T)

    w0 = to_T(net_in[N : 2 * N])
    iext = to_T(net_in[:N])
    return {
        "jt": jt_pad,
        "w0": w0,
        "iext": iext,
        "s0": np.full((1, 1), 0.005, dtype=np.float32),
        "ones_col": np.ones((128, 1), dtype=np.float32),
        "one11": np.ones((1, 1), dtype=np.float32),
    }


def kernel(net_in: np.ndarray, J: np.ndarray, _trace: bool = False):
    if "nc" not in _CACHE:
        _CACHE["nc"] = _build_bass()
    nc = _CACHE["nc"]

    in_map = _prep_inputs(net_in, J)
    core_ids = list(range(8))
    res = run_bass_kernel_spmd(
        nc, [dict(in_map) for _ in core_ids], core_ids, trace=_trace
    )
    _CACHE["last_result"] = res
    out = res.results[0]

    u_t = np.asarray(out["u_out"])  # [128, 14]
    rec_sum = np.float32(np.asarray(out["sum_out"])[0, 0])

    U = u_t.T.reshape(KPAD)[:N].reshape(R, C)
    sq = np.float32(0.04) * U * U
    r_last = sq / rec_sum
    return U, rec_sum, r_last


# revision 38
# speedup vs baseline: 2.4254x; 2.4254x over previous
"""Trainium2 Bass kernel for the CANN recurrent-net problem.

Computation (reference): 14 sequential steps of
    temp   = J @ r                      (J: 1680x1680 fp32)
    U      = temp + Iext
    sq     = (0.2*U)^2 = 0.04*U^2
    recSum = sum(0.005*sq)
    r_new  = sq / recSum
returning (U_last, recSum_last, r_last).

Kernel strategy (single-core latency problem; replicated across the 8 cores
per the sharding hint -- a per-step 8-core allgather floor of ~4.6us x 14
steps would cost more than the whole replicated compute):
  * Keep J'^T = (200*J).T resident in SBUF (zero-padded to [1792, 2048]).
  * Deferred normalization: propagate w = U^2 (unnormalized) as the state.
    J @ r_new = (J' @ w) * (1/sum(w)) by linearity; the 1/sum(w) scale is
    folded into the transpose matmuls as a diag(rinv) rhs. Initial state
    w0 = r0 with scale 0.005 (cancels the 200 baked into J').
  * Matvec on the PE with 4-way column tiling: the 4 N-chunks of 512 run
    at tile_position (0,32c) with k-outer issue order, so 4 streams are
    in flight concurrently (HW-measured 3.0x over sequential; fp32
    matmuls are 2 half-rate HW passes, so this matters).
  * psum chunks (partitions {0,32,64,96}) -> plain copies to sbuf temp
    (2 on ACT, 2 on DVE, concurrently) -> 4 batched transpose matmuls
    (lhsT = strided-partition [4,128] slice, rhs = diag(rinv) [4,4])
    -> [128,16] partition layout, scaled.
  * Elementwise in [128,14] layout: U_T = psum_T + Iext_T (DVE add),
    w = U_T^2 with fused per-partition sum (ACT Square + accum_out;
    the DVE tensor_tensor_reduce equivalent crashes the device),
    128->4 partition reduce via ones[128,4]-matmul (result replicated
    on partitions 0-3), reciprocal on DVE, diag4 = I4 * rinv4 (DVE
    tensor_scalar with per-partition scalar).

Device outputs: U_T [128,14] and recSum [1,1]; the host derives
r_last = 0.04*U^2/recSum (1680 elements, negligible) and reshapes.
Padding: rows/cols >= 1680 are zero in J', Iext and w0, so padded lanes
stay exactly zero through every step and never affect sums.

HW pitfalls baked in (bisected on silicon):
  * is_transpose matmuls after fp32 matmuls -> NRT_EXEC_UNIT_UNRECOVERABLE;
    use regular outer-product matmuls instead.
  * DVE tensor_tensor_reduce -> same crash; use ACT Square + accum_out.
  * PE LDWEIGHTS HW slot fits ONE sync wait; "prime" each DMA-loaded
    constant with a single-dep dummy matmul before compute-dependent uses
    (plus Bacc's generate_event_semaphores legalization for the rest).
"""

import sys

if "/opt/trn_rl_repo" not in sys.path:
    sys.path.insert(0, "/opt/trn_rl_repo")

import numpy as np

import concourse.bacc as bacc
import concourse.mybir as mybir
import concourse.tile as tile
from concourse.bass_utils import run_bass_kernel_spmd

R, C = 30, 56
N = R * C  # 1680
KT = 14  # K tiles of 128 (1792 padded)
NT = 16  # N transpose columns of 128 (2048 padded)
KPAD = KT * 128  # 1792
NPAD = NT * 128  # 2048
STEPS = 14  # PRCN - 1
NCHUNK = 512  # psum bank = 512 fp32
NCH = NPAD // NCHUNK  # 4 column-tiled chunks
F32 = mybir.dt.float32

_CACHE = {}


def _build_bass():
    nc = bacc.Bacc(
        "TRN2", target_bir_lowering=False, debug=False, num_devices=8
    )

    jt = nc.dram_tensor("jt", [KT, 128, NPAD], F32, kind="ExternalInput")
    w0 = nc.dram_tensor("w0", [128, KT], F32, kind="ExternalInput")
    iext = nc.dram_tensor("iext", [128, KT], F32, kind="ExternalInput")
    s0 = nc.dram_tensor("s0", [97, 1], F32, kind="ExternalInput")  # 0.005
    i97 = nc.dram_tensor("i97", [97, 4], F32, kind="ExternalInput")
    ones_col = nc.dram_tensor("ones_col", [128, 97], F32, kind="ExternalInput")

    u_out = nc.dram_tensor("u_out", [128, KT], F32, kind="ExternalOutput")
    sum_out = nc.dram_tensor("sum_out", [1, 1], F32, kind="ExternalOutput")

    with tile.TileContext(nc) as tc:
        with (
            tc.tile_pool(name="jpool", bufs=1) as jpool,
            tc.tile_pool(name="sm", bufs=1) as sm,
            tc.tile_pool(name="ps", bufs=1, space="PSUM") as ps,
        ):
            jt_sb = jpool.tile([128, KT, NPAD], F32)
            temp = sm.tile([128, NCHUNK], F32)  # chunk c on partition 32c
            w = sm.tile([128, KT], F32)
            u_sb = sm.tile([128, KT], F32)
            iext_sb = sm.tile([128, KT], F32)
            rinv97 = sm.tile([97, 1], F32)  # 1/sum(w) on partitions 0..96
            i97_sb = sm.tile([97, 4], F32)  # row 32c = e_c, else zero
            partial = sm.tile([128, 1], F32)
            ones_col_sb = sm.tile([128, 97], F32)
            sums_sb = sm.tile([1, 1], F32)

            psum_mv = ps.tile([128, NCHUNK], F32)  # chunk c at partition 32c
            psum_t = ps.tile([128, NT], F32)  # transposed, scaled result
            psum_s = ps.tile([97, 1], F32)  # sum(w) on partitions 0..96
            psum_x = ps.tile([1, 1], F32)  # scratch for PE priming

            # Input loads (per-K-tile so step-1 matmuls can start early).
            for k in range(KT):
                nc.sync.dma_start(jt_sb[:, k, :], jt[k])
            nc.sync.dma_start(w[:], w0[:])
            nc.sync.dma_start(iext_sb[:], iext[:])
            nc.sync.dma_start(rinv97[:], s0[:])
            nc.sync.dma_start(i97_sb[:], i97[:])
            nc.sync.dma_start(ones_col_sb[:], ones_col[:])

            # temp lanes outside {0,32,64,96} are never written but ARE
            # contracted over (times exact zeros of i97); zero them once
            # so no stale NaN/Inf bit patterns can poison 0*x.
            nc.vector.memset(temp[:, :], 0.0)

            # PE priming (see module docstring).
            for lhsT, rhs in (
                (w[:, 0:1], w[:, 0:1]),
                (ones_col_sb[:, 0:1], ones_col_sb[:, 0:1]),
                (i97_sb[0:1, 0:1], i97_sb[0:1, 0:1]),
            ):
                nc.tensor.matmul(
                    psum_x[0:1, 0:1], lhsT, rhs, start=True, stop=True
                )

            for step in range(STEPS):
                # ---- matvec: 4 column-tiled streams, k-outer issue ----
                for k in range(KT):
                    for c in range(NCH):
                        cs = c * NCHUNK
                        nc.tensor.matmul(
                            psum_mv[32 * c : 32 * c + 1, :],
                            w[:, k : k + 1],
                            jt_sb[:, k, cs : cs + NCHUNK],
                            start=(k == 0),
                            stop=(k == KT - 1),
                            tile_position=(0, 32 * c),
                        )

                # ---- psum -> sbuf chunk copies with exact 1/sum(w)
                # scale (2 ACT + 2 DVE, native lanes 32c -- engines
                # cannot shift partitions). The scale must NOT ride the
                # transpose matmuls: PE fp32 multiplies are lower-
                # precision and push final rel-err past the 2e-2 gate.
                for c in range(NCH):
                    src = psum_mv[32 * c : 32 * c + 1, :]
                    dst = temp[32 * c : 32 * c + 1, :]
                    sc = rinv97[32 * c : 32 * c + 1, 0:1]
                    nc.scalar.activation(
                        dst, src,
                        mybir.ActivationFunctionType.Copy, scale=sc,
                    )

                # ---- transpose: 4 batched K=97 matmuls, NO tile_position
                # (explicit row/col tile positions degrade fp32 matmul
                # precision ~5e-4 on silicon, enough to blow the 2e-2
                # gate after 14 squaring steps). lhsT spans contiguous
                # partitions 0..96; rows between the 4 data lanes are
                # exact zeros times i97's zero rows. Block i fills w
                # columns {i, 4+i, 8+i, 12+i}.
                for i in range(4):
                    nc.tensor.matmul(
                        psum_t[:, i : NT : 4],
                        temp[0:97, i * 128 : (i + 1) * 128],
                        i97_sb[:, :],
                        start=True,
                        stop=True,
                    )

                # ---- U = psum_t + Iext ; w = U^2 ; partial = sum_free ----
                nc.vector.tensor_add(u_sb[:, :], psum_t[:, :KT], iext_sb[:, :])
                nc.scalar.activation(
                    w[:, :],
                    u_sb[:, :],
                    mybir.ActivationFunctionType.Square,
                    accum_out=partial[:, :],
                )

                # ---- sum(w) replicated on partitions 0..96 -> 1/x ----
                nc.tensor.matmul(
                    psum_s[0:97, 0:1], ones_col_sb[:, :], partial[:, :],
                    start=True, stop=True,
                )
                nc.vector.reciprocal(rinv97[0:97, :], psum_s[0:97, :])

            # ---- epilogue: recSum = 2e-4 * sum(w_last) ----
            nc.scalar.activation(
                sums_sb[0:1, :], psum_s[0:1, :],
                mybir.ActivationFunctionType.Copy, scale=2.0e-4,
            )

            nc.sync.dma_start(u_out[:], u_sb[:, :])
            nc.sync.dma_start(sum_out[:], sums_sb[:, :])

    if not nc.is_finalized():
        nc.finalize()
    return nc


def _prep_inputs(net_in: np.ndarray, J: np.ndarray):
    net_in = np.asarray(net_in, dtype=np.float32)
    J = np.asarray(J, dtype=np.float32)

    jt_pad = np.zeros((KPAD, NPAD), dtype=np.float32)
    jt_pad[:N, :N] = (np.float32(200.0) * J).T
    jt_pad = np.ascontiguousarray(jt_pad.reshape(KT, 128, NPAD))

    def to_T(v):
        p = np.zeros(KPAD, dtype=np.float32)
        p[:N] = v
        return np.ascontiguousarray(p.reshape(KT, 128).T)

    w0 = to_T(net_in[N : 2 * N])
    iext = to_T(net_in[:N])
    i97 = np.zeros((97, 4), dtype=np.float32)
    for c in range(4):
        i97[32 * c, c] = 1.0
    return {
        "jt": jt_pad,
        "w0": w0,
        "iext": iext,
        "s0": np.full((97, 1), 0.005, dtype=np.float32),
        "i97": i97,
        "ones_col": np.ones((128, 97), dtype=np.float32),
    }


def kernel(net_in: np.ndarray, J: np.ndarray, _trace: bool = False):
    if "nc" not in _CACHE:
        _CACHE["nc"] = _build_bass()
    nc = _CACHE["nc"]

    in_map = _prep_inputs(net_in, J)
    core_ids = list(range(8))
    res = run_bass_kernel_spmd(
        nc, [dict(in_map) for _ in core_ids], core_ids, trace=_trace
    )
    _CACHE["last_result"] = res
    out = res.results[0]

    u_t = np.asarray(out["u_out"])  # [128, 14]
    rec_sum = np.float32(np.asarray(out["sum_out"])[0, 0])

    U = u_t.T.reshape(KPAD)[:N].reshape(R, C)
    sq = np.float32(0.04) * U * U
    r_last = sq / rec_sum
    return U, rec_sum, r_last
